# revision 15
# baseline (speedup 1.0000x reference)
"""DN4 retrieval kernel for 8 Trainium2 NeuronCores (nn_DN4_10668698763884).

Pipeline (reference: 4x [conv3x3 -> batch-stat BN -> LeakyReLU(0.2) -> pool?]
encoder, then cosine sim between query/support local descriptors, top-3 over
support descriptors, summed over descriptors and k).

BatchNorm uses batch statistics over the *whole* encode batch (all 32 query
images jointly / all 50 support images jointly), which couples every image at
each layer.  We therefore run 5 SPMD launches with a host-side reduction of
per-image BN partial statistics between launches:

  L0: conv1 (host im2col, K=54 block-diag image pairs)     -> y1 + stats
  L1: BN1+LReLU+pool + conv2 (9 taps, K=128 pairs)         -> y2 + stats
  L2: BN2+LReLU+pool + conv3                               -> y3 + stats
  L3: BN3+LReLU + conv4                                    -> y4 + stats
  L4: BN4+LReLU, l2-normalize, cosine sim (f32r matmul),
      top-3 via DVE max8 on PSUM, masked final reduction   -> scores

Sharding: data-parallel over images.  32 query images -> 4/core; 50 support
images -> padded to 56 -> 7/core; +1 dummy image pads each core to 12 images
(6 pairs) so every core runs an identical program.  The similarity stage is
data-parallel over (episode, query): core c scores queries 4c..4c+3 against
its episode's 25 support images (replicated to the 4 cores of that episode).
"""

import numpy as np
import jax
import jax.numpy as jnp
from jax.experimental.shard_map import shard_map
from jax.sharding import Mesh, PartitionSpec

import concourse.bass as bass
import concourse.mybir as mybir
import concourse.tile as tile
from concourse import bass2jax

AF = mybir.ActivationFunctionType
ALU = mybir.AluOpType
F32 = mybir.dt.float32
F32R = mybir.dt.float32r

B, NQ, WAY, SHOT = 2, 16, 5, 5
CIN, HW0 = 3, 84
D = 64
KTOP = 3
BN_EPS = 1e-5
SLOPE = 0.2
NCORES = 8
NQL = 4            # query images per core
NSL = 7            # support slots per core (50 -> 56)
NIMG = 12          # images per core incl. 1 dummy (6 pairs)
NPAIR = 6
L = 21 * 21        # 441 descriptors per image
M = SHOT * L       # 2205 descriptors per class
LALL = NQL * L     # 1764 query descriptors per core
NLBLK = 14         # ceil(1764 / 128)


def _legalize_waits(nc):
    """This container's walrus accepts at most 1 sem-wait per instruction
    (2 on EventSemaphore).  Tile attaches multi-waits; hoist extras onto
    EventSemaphore carriers inserted just before, on the same engine."""
    for f in nc.m.functions:
        for bb in f.blocks:
            insts = list(bb.instructions)
            out, changed = [], False
            for inst in insts:
                si = inst.sync_info
                waits = list(si.on_wait) if si is not None else []
                cap = 2 if inst.opcode == 'EventSemaphore' else 1
                if len(waits) > cap:
                    changed = True
                    extras, keep = waits[:-cap], waits[-cap:]
                    for i in range(0, len(extras), 2):
                        ev = mybir.InstEventSemaphore(
                            name=f"{inst.name}-wc{i}", ins=[], outs=[],
                            engine=inst.engine)
                        if ev.sync_info is None:
                            ev.sync_info = mybir.SyncInfo(
                                on_wait=extras[i:i + 2], on_update=[])
                        else:
                            ev.sync_info.on_wait = extras[i:i + 2]
                        out.append(ev)
                    si.on_wait = keep
                out.append(inst)
            if changed:
                bb.instructions = out
    return nc


# ---------------------------------------------------------------- L0: conv1
def _build_l0():
    S = HW0                      # 84
    SPAT = S * S                 # 7056
    NCH = 14                     # spatial chunks of 6 rows (504 cols)
    CHW = SPAT // NCH            # 504
    nc = bass.Bass(name="dn4_l0")
    xcol = nc.dram_tensor("xcol", [NIMG, 27, SPAT], F32R, kind="ExternalInput")
    w = nc.dram_tensor("w", [54, 128], F32R, kind="ExternalInput")
    y = nc.dram_tensor("y", [NIMG, D, SPAT], F32, kind="ExternalOutput")
    st = nc.dram_tensor("st", [NPAIR, 128, 2], F32, kind="ExternalOutput")

    with tile.TileContext(nc) as tc:
        with tc.tile_pool(name="wp", bufs=1) as wp, \
             tc.tile_pool(name="sb", bufs=2) as sb, \
             tc.tile_pool(name="ps", bufs=6, space="PSUM") as ps:
            wt = wp.tile([54, 128], F32R)
            nc.sync.dma_start(wt[:], w[:])
            for p in range(NPAIR):
                xt = sb.tile([54, SPAT], F32R, name="xt")
                nc.sync.dma_start(
                    xt[:], xcol[2 * p:2 * p + 2].rearrange("a b c -> (a b) c"))
                ysb = sb.tile([128, SPAT], F32, name="ysb")
                stt = sb.tile([128, NCH, 6], F32, name="stt")
                for ch in range(NCH):
                    sl = slice(ch * CHW, (ch + 1) * CHW)
                    pt = ps.tile([128, CHW], F32, name="pt")
                    nc.tensor.matmul(pt[:], wt[:], xt[:, sl],
                                     start=True, stop=True)
                    nc.scalar.copy(ysb[:, sl], pt[:])
                    nc.vector.bn_stats(stt[:, ch, :], pt[:])
                sa = sb.tile([128, 2], F32, name="sa")
                nc.vector.bn_aggr(sa[:], stt[:])
                nc.sync.dma_start(st[p], sa[:])
                nc.sync.dma_start(
                    y[2 * p:2 * p + 2].rearrange("a b c -> (a b) c"), ysb[:])
    return _legalize_waits(nc)


# ------------------------------------------------- L1/L2: BN+pool+conv layer
def _build_pool_conv(name, s_in):
    """BN+LReLU at s_in x s_in, maxpool2 -> s x s, conv3x3 -> y + stats."""
    s = s_in // 2
    spat_in = s_in * s_in
    spat = s * s
    # output-row chunks, each <= 512 columns and >= 256 (f32r full rate)
    rows = []
    r = s
    while r > 0:
        take = min(512 // s, r)
        rows.append(take)
        r -= take
    nc = bass.Bass(name=name)
    yp = nc.dram_tensor("yp", [NIMG, D, spat_in], F32, kind="ExternalInput")
    bnsb = nc.dram_tensor("bnsb", [NIMG, D, 2], F32, kind="ExternalInput")
    w = nc.dram_tensor("w", [128, 9, 128], F32R, kind="ExternalInput")
    y = nc.dram_tensor("y", [NIMG, D, spat], F32, kind="ExternalOutput")
    st = nc.dram_tensor("st", [NPAIR, 128, 2], F32, kind="ExternalOutput")

    with tile.TileContext(nc) as tc:
        with tc.tile_pool(name="wp", bufs=1) as wp, \
             tc.tile_pool(name="sb", bufs=2) as sb, \
             tc.tile_pool(name="ps", bufs=6, space="PSUM") as ps:
            wts = wp.tile([128, 9, 128], F32R)
            nc.sync.dma_start(wts[:], w[:])
            for p in range(NPAIR):
                yt = sb.tile([128, spat_in], F32, name="yt")
                nc.sync.dma_start(
                    yt[:], yp[2 * p:2 * p + 2].rearrange("a b c -> (a b) c"))
                sbt = sb.tile([128, 2], F32, name="sbt")
                nc.sync.dma_start(
                    sbt[:], bnsb[2 * p:2 * p + 2].rearrange("a b c -> (a b) c"))
                z = sb.tile([128, spat_in], F32, name="z")
                nc.scalar.activation(z[:], yt[:], AF.Prelu,
                                     bias=sbt[:, 1:2], scale=sbt[:, 0:1],
                                     alpha=SLOPE)
                z4 = z.rearrange("p (h w2 two) -> p h w2 two", two=2,
                                 h=s_in, w2=s)
                ph = sb.tile([128, s_in, s], F32, name="ph")
                nc.vector.tensor_tensor(ph[:], z4[:, :, :, 0], z4[:, :, :, 1],
                                        ALU.max)
                # rhs inner count must be even for f32r matmuls; pad the
                # conv window width to wv (s or s+1) with zero border cols
                wv = s if s % 2 == 0 else s + 1
                pad = sb.tile([128, s + 2, wv + 2], F32R, name="pad")
                nc.vector.memset(pad[:].bitcast(F32), 0.0)
                ph4 = ph.rearrange("p (h2 two) w -> p h2 two w", two=2, h2=s)
                nc.vector.tensor_tensor(pad[:, 1:s + 1, 1:s + 1],
                                        ph4[:, :, 0, :], ph4[:, :, 1, :],
                                        ALU.max)
                ysb = sb.tile([128, spat], F32, name="ysb")
                ysb3 = ysb.rearrange("p (h w) -> p h w", h=s)
                ngrp = len(rows) if s % 2 == 0 else 1
                stt = sb.tile([128, ngrp, 6], F32, name="stt")
                r0 = 0
                for ci, nr in enumerate(rows):
                    pt = ps.tile([128, nr * wv], F32, name="pt")
                    pt3 = pt.rearrange("p (h w) -> p h w", h=nr)
                    t = 0
                    for dy in range(3):
                        for dx in range(3):
                            nc.tensor.matmul(
                                pt3[:],
                                wts[:, dy * 3 + dx, :],
                                pad[:, r0 + dy:r0 + dy + nr, dx:dx + wv],
                                start=(t == 0), stop=(t == 8))
                            t += 1
                    nc.scalar.copy(ysb3[:, r0:r0 + nr, :], pt3[:, :, :s])
                    if s % 2 == 0:
                        nc.vector.bn_stats(stt[:, ci, :], pt[:])
                    r0 += nr
                if s % 2 == 1:
                    nc.vector.bn_stats(stt[:, 0, :], ysb[:])
                sa = sb.tile([128, 2], F32, name="sa")
                nc.vector.bn_aggr(sa[:], stt[:])
                nc.sync.dma_start(st[p], sa[:])
                nc.sync.dma_start(
                    y[2 * p:2 * p + 2].rearrange("a b c -> (a b) c"), ysb[:])
    return _legalize_waits(nc)


# ---------------------------------------------------- L3: BN + conv4 (21x21)
def _build_l3():
    s = 21
    spat = s * s
    nc = bass.Bass(name="dn4_l3")
    yp = nc.dram_tensor("yp", [NIMG, D, spat], F32, kind="ExternalInput")
    bnsb = nc.dram_tensor("bnsb", [NIMG, D, 2], F32, kind="ExternalInput")
    w = nc.dram_tensor("w", [128, 9, 128], F32R, kind="ExternalInput")
    y = nc.dram_tensor("y", [NIMG, D, spat], F32, kind="ExternalOutput")
    st = nc.dram_tensor("st", [NPAIR, 128, 2], F32, kind="ExternalOutput")

    with tile.TileContext(nc) as tc:
        with tc.tile_pool(name="wp", bufs=1) as wp, \
             tc.tile_pool(name="sb", bufs=2) as sb, \
             tc.tile_pool(name="ps", bufs=6, space="PSUM") as ps:
            wts = wp.tile([128, 9, 128], F32R)
            nc.sync.dma_start(wts[:], w[:])
            for p in range(NPAIR):
                yt = sb.tile([128, spat], F32, name="yt")
                nc.sync.dma_start(
                    yt[:], yp[2 * p:2 * p + 2].rearrange("a b c -> (a b) c"))
                sbt = sb.tile([128, 2], F32, name="sbt")
                nc.sync.dma_start(
                    sbt[:], bnsb[2 * p:2 * p + 2].rearrange("a b c -> (a b) c"))
                wv = s + 1                      # even conv window width
                pad = sb.tile([128, s + 2, wv + 2], F32R, name="pad")
                nc.vector.memset(pad[:].bitcast(F32), 0.0)
                yt3 = yt.rearrange("p (h w) -> p h w", h=s)
                nc.scalar.activation(pad[:, 1:s + 1, 1:s + 1], yt3[:],
                                     AF.Prelu, bias=sbt[:, 1:2],
                                     scale=sbt[:, 0:1], alpha=SLOPE)
                ysb = sb.tile([128, spat], F32, name="ysb")
                stt = sb.tile([128, 1, 6], F32, name="stt")
                pt = ps.tile([128, s * wv], F32, name="pt")
                pt3 = pt.rearrange("p (h w) -> p h w", h=s)
                t = 0
                for dy in range(3):
                    for dx in range(3):
                        nc.tensor.matmul(
                            pt3[:], wts[:, dy * 3 + dx, :],
                            pad[:, dy:dy + s, dx:dx + wv],
                            start=(t == 0), stop=(t == 8))
                        t += 1
                ysb3 = ysb.rearrange("p (h w) -> p h w", h=s)
                nc.scalar.copy(ysb3[:], pt3[:, :, :s])
                nc.vector.bn_stats(stt[:, 0, :], ysb[:])
                sa = sb.tile([128, 2], F32, name="sa")
                nc.vector.bn_aggr(sa[:], stt[:])
                nc.sync.dma_start(st[p], sa[:])
                nc.sync.dma_start(
                    y[2 * p:2 * p + 2].rearrange("a b c -> (a b) c"), ysb[:])
    return _legalize_waits(nc)


# ------------------------------------- L4: BN4, l2norm, sim, top-3, scores
def _build_l4():
    NS = WAY * SHOT              # 25 support images per episode
    MS = NS * L                  # 11025 support descriptors
    nc = bass.Bass(name="dn4_l4")
    qf = nc.dram_tensor("qf", [D, LALL], F32, kind="ExternalInput")
    sf = nc.dram_tensor("sf", [D, MS], F32, kind="ExternalInput")
    bnq = nc.dram_tensor("bnq", [D, 2], F32, kind="ExternalInput")
    bns = nc.dram_tensor("bns", [D, 2], F32, kind="ExternalInput")
    ones = nc.dram_tensor("ones", [D, D], F32, kind="ExternalInput")
    selm = nc.dram_tensor("selm", [128, NLBLK, NQL], F32, kind="ExternalInput")
    scores = nc.dram_tensor("scores", [NQL, WAY], F32, kind="ExternalOutput")

    with tile.TileContext(nc) as tc:
        with tc.tile_pool(name="cst", bufs=1) as cst, \
             tc.tile_pool(name="sb", bufs=2) as sb, \
             tc.tile_pool(name="mx", bufs=4) as mxp, \
             tc.tile_pool(name="ps", bufs=1, space="PSUM") as ps, \
             tc.tile_pool(name="pn", bufs=2, space="PSUM") as pn, \
             tc.tile_pool(name="pf", bufs=1, space="PSUM") as pf:
            onest = cst.tile([D, D], F32)
            nc.sync.dma_start(onest[:], ones[:])
            selt = cst.tile([128, NLBLK, NQL], F32)
            nc.sync.dma_start(selt[:], selm[:])
            bnqt = cst.tile([D, 2], F32)
            nc.sync.dma_start(bnqt[:], bnq[:])
            bnst = cst.tile([D, 2], F32)
            nc.sync.dma_start(bnst[:], bns[:])

            def normalize(src_dram, n_col, bn_t, tag):
                act = cst.tile([D, n_col], F32, name=f"act_{tag}")
                nc.sync.dma_start(act[:], src_dram[:])
                nc.scalar.activation(act[:], act[:], AF.Prelu,
                                     bias=bn_t[:, 1:2], scale=bn_t[:, 0:1],
                                     alpha=SLOPE)
                nrm = cst.tile([D, n_col], F32, name=f"nrm_{tag}")
                for c0 in range(0, n_col, L):
                    sqc = sb.tile([D, L + 1], F32, name="sqc")
                    nc.vector.memset(sqc[:, L:], 0.0)
                    nc.vector.tensor_tensor(sqc[:, :L], act[:, c0:c0 + L],
                                            act[:, c0:c0 + L], ALU.mult)
                    pnorm = pn.tile([D, L + 1], F32, name="pnorm")
                    nc.tensor.matmul(pnorm[:], onest[:], sqc[:],
                                     start=True, stop=True)
                    nc.scalar.sqrt(nrm[:, c0:c0 + L], pnorm[:, :L])
                nc.vector.tensor_scalar_max(nrm[:], nrm[:], 1e-12)
                nc.vector.reciprocal(nrm[:], nrm[:])
                out = cst.tile([D, n_col + 4], F32R, name=f"n_{tag}")
                nc.vector.memset(out[:, n_col:].bitcast(F32), 0.0)
                nc.vector.tensor_tensor(out[:, :n_col], act[:], nrm[:],
                                        ALU.mult)
                return out

            qn = normalize(qf, LALL, bnqt, "q")
            sn = normalize(sf, MS, bnst, "s")

            WAYP = 6                      # padded for even-N final matmul
            s_all = cst.tile([128, WAYP, NLBLK], F32)
            nc.vector.memset(s_all[:], 0.0)

            # Similarity + top-3: per (way, L-block), fill two PSUM tiles
            # with bank-aligned <=512-wide matmuls, then only 2 InstMax
            # passes (instead of 5) amortize the DVE PSUM-read overhead.
            # Sub-matmul widths must be even (f32r); tileB's last chunk
            # reads 2 cols past the way block, excluded from the max input.
            for wy in range(WAY):
                for bk in range(NLBLK):
                    pb = min(128, LALL - bk * 128)   # 128 or 100
                    max8 = mxp.tile([128, 16], F32, name="max8")
                    ptA = ps.tile([128, 1536], F32, name="simpA")
                    ptB = ps.tile([128, 672], F32, name="simpB")
                    qs = qn[:, bk * 128:bk * 128 + pb]
                    for j, (dst, off, wdt) in enumerate(
                            [(ptA, 0, 512), (ptA, 512, 512), (ptA, 1024, 512),
                             (ptB, 0, 512), (ptB, 512, 160)]):
                        base = wy * M + (0 if dst is ptA else 1536) + off
                        nc.tensor.matmul(
                            dst[:pb, off:off + wdt], qs,
                            sn[:, base:base + wdt], start=True, stop=True)
                    nc.vector.max(max8[:pb, 0:8], ptA[:pb, :])
                    nc.vector.max(max8[:pb, 8:16], ptB[:pb, :M - 1536])
                    top8 = mxp.tile([128, 8], F32, name="top8")
                    nc.vector.max(top8[:pb], max8[:pb, :])
                    nc.vector.reduce_sum(s_all[:pb, wy, bk:bk + 1],
                                         top8[:pb, 0:KTOP],
                                         axis=mybir.AxisListType.X)

            psc = pf.tile([NQL, WAYP], F32)
            for bk in range(NLBLK):
                nc.tensor.matmul(psc[:], selt[:, bk, :], s_all[:, :, bk],
                                 start=(bk == 0), stop=(bk == NLBLK - 1))
            osc = sb.tile([NQL, WAYP], F32, name="osc")
            nc.scalar.copy(osc[:], psc[:])
            nc.sync.dma_start(scores[:], osc[:, :WAY])
    return _legalize_waits(nc)


_programs = {}


def _get_programs():
    if not _programs:
        _programs["l0"] = _build_l0()
        _programs["l1"] = _build_pool_conv("dn4_l1", 84)
        _programs["l2"] = _build_pool_conv("dn4_l2", 42)
        _programs["l3"] = _build_l3()
        _programs["l4"] = _build_l4()
    return _programs


# ------------------------------------------------------------- host helpers
def _im2col(imgs):
    """[n, 3, 84, 84] -> [n, 27, 7056] with partition order (tap*3 + channel)."""
    n = imgs.shape[0]
    xp = np.zeros((n, CIN, HW0 + 2, HW0 + 2), np.float32)
    xp[:, :, 1:HW0 + 1, 1:HW0 + 1] = imgs
    cols = np.empty((n, 9, CIN, HW0, HW0), np.float32)
    for t in range(9):
        dy, dx = t // 3, t % 3
        cols[:, t] = xp[:, :, dy:dy + HW0, dx:dx + HW0]
    return cols.reshape(n, 27, HW0 * HW0)


def _blockdiag(a):
    k, m = a.shape
    out = np.zeros((2 * k, 2 * m), np.float32)
    out[:k, :m] = a
    out[k:, m:] = a
    return out


def _group_stats(means, varis):
    """Exact batch stats from equal-size per-image (mean, var) in float64."""
    mu = means.mean(axis=0)
    ex2 = (varis + means ** 2).mean(axis=0)
    return mu, ex2 - mu ** 2


_MESH = None
_SHARD = None


def _get_shard():
    global _MESH, _SHARD
    if _SHARD is None:
        _MESH = Mesh(np.asarray(jax.devices()[:NCORES]), ("core",))
        _SHARD = jax.sharding.NamedSharding(_MESH, PartitionSpec("core"))
    return _SHARD


class _Runner:
    """Compiled SPMD executor for one Bass program; the jax.jit function is
    built once so repeated calls hit the executable cache (no retrace /
    recompile / NEFF reload per call, unlike run_bass_kernel_spmd)."""

    def __init__(self, nc):
        bass2jax.install_neuronx_cc_hook()
        self.nc = nc
        partition_name = (nc.partition_id_tensor.name
                          if nc.partition_id_tensor else None)
        in_names, out_names, out_avals = [], [], []
        for alloc in nc.m.functions[0].allocations:
            if not isinstance(alloc, mybir.MemoryLocationSet):
                continue
            name = alloc.memorylocations[0].name
            if alloc.kind == "ExternalInput":
                if name != partition_name:
                    in_names.append(name)
            elif alloc.kind == "ExternalOutput":
                shape = tuple(alloc.tensor_shape)
                out_avals.append(jax.core.ShapedArray(
                    shape, mybir.dt.np(alloc.dtype)))
                out_names.append(name)
        self.in_names = list(in_names)
        self.out_names = list(out_names)
        n_params = len(in_names)
        all_in = in_names + out_names + (
            [partition_name] if partition_name else [])
        self.out_shapes = [(a.shape, a.dtype) for a in out_avals]

        def _body(*args):
            operands = list(args)
            if partition_name is not None:
                operands.append(bass2jax.partition_id_tensor())
            outs = bass2jax._bass_exec_p.bind(
                *operands,
                out_avals=tuple(out_avals),
                in_names=tuple(all_in),
                out_names=tuple(out_names),
                lowering_input_output_aliases=(),
                sim_require_finite=True,
                sim_require_nnan=True,
                nc=nc,
            )
            return tuple(outs)

        self._shard = _get_shard()
        n_outs = len(out_names)
        inner = shard_map(
            _body, mesh=_MESH,
            in_specs=(PartitionSpec("core"),) * (n_params + n_outs),
            out_specs=(PartitionSpec("core"),) * n_outs,
            check_rep=False)

        self.fn = jax.jit(inner, out_shardings=(self._shard,) * n_outs)
        self._zeros = [jax.device_put(np.zeros((NCORES * s[0], *s[1:]), d),
                                      self._shard)
                       for s, d in self.out_shapes]

    def __call__(self, global_inputs):
        args = []
        for n in self.in_names:
            x = global_inputs[n]
            if not (isinstance(x, jax.Array) and x.sharding == self._shard):
                x = jax.device_put(x, self._shard)
            args.append(x)
        outs = self.fn(*args, *self._zeros)
        return dict(zip(self.out_names, outs))


_runners = {}


def _run(key, global_inputs):
    if key not in _runners:
        _runners[key] = _Runner(_get_programs()[key])
    return _runners[key](global_inputs)


# ------------------------------------------------ device-side (jnp) glue
# per-core image table: [4 query, 7 support slots, 1 dummy]; static.
CORE_IMGS = []
for _c in range(NCORES):
    _rows = [(0, 4 * _c + i) for i in range(NQL)]
    for _s in range(7 * _c, 7 * _c + NSL):
        _rows.append((1, _s) if _s < B * WAY * SHOT else (2, -1))
    _rows.append((2, -1))
    CORE_IMGS.append(_rows)

_Q_ROWS = [c * NIMG + i for c in range(NCORES)
           for i, (k, _) in enumerate(CORE_IMGS[c]) if k == 0]
_S_ROWS = [c * NIMG + i for c in range(NCORES)
           for i, (k, _) in enumerate(CORE_IMGS[c]) if k == 1]
_IS_Q = np.array([1.0 if CORE_IMGS[c][i][0] == 0 else 0.0
                  for c in range(NCORES) for i in range(NIMG)],
                 np.float32)[:, None, None]
# global image slot for each (kind, idx): query gi -> slot, support gi -> slot
_Q_SLOT = {}
_S_SLOT = {}
for _c in range(NCORES):
    for _i, (_k, _gi) in enumerate(CORE_IMGS[_c]):
        if _k == 0:
            _Q_SLOT[_gi] = _c * NIMG + _i
        elif _k == 1:
            _S_SLOT[_gi] = _c * NIMG + _i


def _g_im2col(raw):
    """[96, 3, 84, 84] -> [96, 27, 7056], tap-major partition order."""
    xp = jnp.pad(raw, ((0, 0), (0, 0), (1, 1), (1, 1)))
    cols = [xp[:, :, dy:dy + HW0, dx:dx + HW0]
            for dy in range(3) for dx in range(3)]
    return jnp.stack(cols, axis=1).reshape(NCORES * NIMG, 27, HW0 * HW0)


def _g_bnsb(st_g, g_l, b_l):
    """st [48, 128, 2] global -> bnsb [96, 64, 2] + tiled [512, 2] (device)."""
    st = st_g.reshape(NCORES * NIMG, D, 2)   # [96, 64, 2] per image
    qm = st[jnp.array(_Q_ROWS), :, 0]
    qv = st[jnp.array(_Q_ROWS), :, 1]
    sm = st[jnp.array(_S_ROWS), :, 0]
    sv = st[jnp.array(_S_ROWS), :, 1]

    def grp(m, v):
        mu = m.mean(axis=0)
        ex2 = (v + m * m).mean(axis=0)
        return mu, ex2 - mu * mu

    mu_q, var_q = grp(qm, qv)
    mu_s, var_s = grp(sm, sv)
    sc_q = g_l / jnp.sqrt(var_q + BN_EPS)
    bi_q = b_l - mu_q * sc_q
    sc_s = g_l / jnp.sqrt(var_s + BN_EPS)
    bi_s = b_l - mu_s * sc_s
    qsb = jnp.stack([sc_q, bi_q], axis=1)       # [64, 2]
    ssb = jnp.stack([sc_s, bi_s], axis=1)
    isq = jnp.asarray(_IS_Q)
    arr = isq * qsb[None] + (1.0 - isq) * ssb[None]   # [96, 64, 2]
    return arr, jnp.tile(qsb, (NCORES, 1)), jnp.tile(ssb, (NCORES, 1))


def _g_route(y4_g):
    """y4 [96, 64, 441] global -> qf [512, 1764], sf [512, 11025]."""
    q_feat = y4_g[jnp.array([_Q_SLOT[g] for g in range(B * NQ)])]
    s_feat = y4_g[jnp.array([_S_SLOT[g] for g in range(B * WAY * SHOT)])]
    qf = q_feat.reshape(NCORES, NQL, D, L).transpose(0, 2, 1, 3) \
               .reshape(NCORES * D, LALL)
    sf_eps = s_feat.reshape(B, 25, D, L).transpose(0, 2, 1, 3) \
                   .reshape(B, D, 25 * L)
    sf = jnp.concatenate([jnp.tile(sf_eps[e], (NQ // NQL, 1))
                          for e in range(B)], axis=0)
    return qf, sf


def _init_glue():
    sh = _get_shard()
    global _g_im2col, _g_bnsb, _g_route
    _g_im2col = jax.jit(_g_im2col, out_shardings=sh)
    _g_bnsb = jax.jit(_g_bnsb, out_shardings=(sh, sh, sh))
    _g_route = jax.jit(_g_route, out_shardings=(sh, sh))


_init_glue_done = False
_dev_cache = {}


def _dev_const(key, builder):
    if key not in _dev_cache:
        _dev_cache[key] = jax.device_put(builder(), _get_shard())
    return _dev_cache[key]


def kernel(query, support, W1, g1, b1, W2, g2, b2, W3, g3, b3, W4, g4, b4):
    _get_programs()
    global _init_glue_done
    if not _init_glue_done:
        _init_glue()
        _init_glue_done = True
    query = np.asarray(query, np.float32)
    support = np.asarray(support, np.float32)
    q_imgs = query.reshape(B * NQ, CIN, HW0, HW0)
    s_imgs = support.reshape(B * WAY * SHOT, CIN, HW0, HW0)

    # raw image layout per core (8.5MB upload; im2col expands on device)
    raw_g = np.zeros((NCORES * NIMG, CIN, HW0, HW0), np.float32)
    for c in range(NCORES):
        for i, (kind, gi) in enumerate(CORE_IMGS[c]):
            if kind == 0:
                raw_g[c * NIMG + i] = q_imgs[gi]
            elif kind == 1:
                raw_g[c * NIMG + i] = s_imgs[gi]

    # static device-cached weights / constants (keyed by input identity)
    wkey = (id(W1), id(W2), id(W3), id(W4))

    def build_w1():
        w1col = np.asarray(W1).transpose(2, 3, 1, 0).reshape(27, D)
        return np.tile(_blockdiag(w1col.astype(np.float32)), (NCORES, 1))

    def build_wl(Wl):
        taps = np.asarray(Wl).transpose(2, 3, 1, 0).reshape(9, D, D)
        wbd = np.ascontiguousarray(
            np.stack([_blockdiag(t.astype(np.float32)) for t in taps], axis=1))
        return np.tile(wbd, (NCORES, 1, 1))

    w1bd_g = _dev_const(("w1", wkey), build_w1)
    w2bd_g = _dev_const(("w2", wkey), lambda: build_wl(W2))
    w3bd_g = _dev_const(("w3", wkey), lambda: build_wl(W3))
    w4bd_g = _dev_const(("w4", wkey), lambda: build_wl(W4))

    def build_selm():
        selm = np.zeros((128, NLBLK, NQL), np.float32)
        for gidx in range(LALL):
            selm[gidx % 128, gidx // 128, gidx // L] = 1.0
        return np.tile(selm, (NCORES, 1, 1))

    selm_g = _dev_const(("selm",), build_selm)
    ones_g = _dev_const(("ones",),
                        lambda: np.tile(np.ones((D, D), np.float32),
                                        (NCORES, 1)))
    gj = [jnp.asarray(np.asarray(g, np.float32)) for g in (g1, g2, g3, g4)]
    bj = [jnp.asarray(np.asarray(b, np.float32)) for b in (b1, b2, b3, b4)]

    # ---- device-side chain (async; only the final scores sync to host)
    xcol_g = _g_im2col(jnp.asarray(raw_g))
    r = _run("l0", {"xcol": xcol_g, "w": w1bd_g})
    bn1, _, _ = _g_bnsb(r["st"], gj[0], bj[0])
    r = _run("l1", {"yp": r["y"], "bnsb": bn1, "w": w2bd_g})
    bn2, _, _ = _g_bnsb(r["st"], gj[1], bj[1])
    r = _run("l2", {"yp": r["y"], "bnsb": bn2, "w": w3bd_g})
    bn3, _, _ = _g_bnsb(r["st"], gj[2], bj[2])
    r = _run("l3", {"yp": r["y"], "bnsb": bn3, "w": w4bd_g})
    _, qsb, ssb = _g_bnsb(r["st"], gj[3], bj[3])

    qf_g, sf_g = _g_route(r["y"].reshape(NCORES * NIMG, D, L))
    r4 = _run("l4", {
        "qf": qf_g, "sf": sf_g, "bnq": qsb, "bns": ssb,
        "ones": ones_g, "selm": selm_g})

    out = np.asarray(r4["scores"]).reshape(B * NQ, WAY)
    return out.astype(np.float32)
